# revision 18
# baseline (speedup 1.0000x reference)
"""Trainium2 kernel for nn_ColorMapGenerator.

Reference semantics (NCHW in / NCHW out):
    x   = img.transpose(0,2,3,1)                 # [B,H,W,3]
    rgb = (x + 1) * 127.5
    idx = (rgb[...,0]*65536 + rgb[...,1]*256 + rgb[...,2]).astype(int32)
    y   = tanh(weight[idx] * x + bias[idx])      # per-pixel LUT rows
    out = y.transpose(0,3,1,2)                   # [B,3,H,W]

The 16.7M-row weight/bias tables are checked on the host:

* every row identical AND the row itself uniform across channels (true
  for this problem's inputs: weight all ones, bias all zeros) -> the op
  is y = tanh(s*x + b) elementwise with scalar s, b.  Channel/layout
  structure is irrelevant, so each core streams its flat 12MB slice of
  the batch through SBUF in [128, c] tiles whose DRAM rows are up to
  16KB contiguous (bigger DMA packets than the 8KB plane rows).
* rows identical but per-channel values differ -> per-channel plane
  kernel (12 [128,2048] planes per core, scale immediate + bias column).
* rows differ -> host numpy fallback (the gather actually matters).

Flat-kernel design (per core, raw Bass):
  - tiles of [128, c] f32, c schedule ramps 1024 -> 4096 -> 1024 so the
    first ACT starts early and the tail drains fine-grained; every tile
    has its own SBUF slot (12MB total) and its own DMA semaphore.
  - in-DMAs are issued from the SP queue up front; ACT gates each tanh
    on the tile's in-sem reaching 16 (all 16 SDMA engines done - a
    single cumulative semaphore would be unsound: a fast engine's
    increments for later DMAs can stand in for a lagging engine's).
  - out-DMAs are issued from ACT's own HWDGE queue right after the
    datapath drain, so SP<->ACT semaphore round-trips are off the
    critical path and two queues keep the SDMA engines fed.
  - SP parks on the final out_sem target so the program ends only when
    every out-DMA completed.
  - walrus in this toolchain encodes at most ONE sync-wait per
    instruction; _split_multi_waits hoists extras onto standalone NoOps.
"""

import numpy as np

B, C, H, W = 32, 3, 512, 512
N_CORES = 8
IMGS_PER_CORE = B // N_CORES           # 4
PLANES_PER_CORE = IMGS_PER_CORE * C    # 12 [128,2048] planes per core
PART = 128
COLS = (H * W) // PART                 # 2048
BUFS = 6

# Flat-path tile schedule: (rows, cols) per tile, sum(rows*cols) must
# equal the per-core element count.  Cols stay powers of two: 8KB/16KB
# packets run at the full ~26.5-27 GB/s per SDMA engine (odd sizes like
# 16320B run at half rate).  A DMA's rows are sprayed over the largest
# divisor of the row count <= 16 SDMA engines (allocated contiguously
# from engine 64): 128 rows -> all 16 engines x 8 rows, 120 rows -> 15
# engines x 8 rows with engine 79 skipped.  Engine 79 also hosts HWDGE
# queue management and accumulates ~10-15% packet-time overhead, so one
# 120-row tile trims its byte share to ~0.91 of the others'.  Small
# head tiles start the ACT pipeline early; the tiny 8-row tile last
# keeps the write-only tail short.
FLAT_TILES = [
    (128, 4096), (128, 4096), (128, 4096), (128, 4096), (128, 4096),
    (120, 2048),
    (16, 4096), (16, 4096),
    (120, 1024),
    (16, 1024), (16, 512),
]
assert sum(r * c for r, c in FLAT_TILES) == IMGS_PER_CORE * C * H * W


def _split_multi_waits(nc, max_waits=1):
    from concourse import mybir

    for fn in nc.m.functions:
        for blk in fn.blocks:
            new_insts = []
            for inst in blk.instructions:
                si = inst.sync_info
                if si is not None and si.on_wait and len(si.on_wait) > max_waits:
                    waits = list(si.on_wait)
                    extra, keep = waits[:-max_waits], waits[-max_waits:]
                    for w in extra:
                        nop = mybir.InstNoOp(
                            name=nc.get_next_instruction_name(),
                            ins=[],
                            outs=[],
                            sync_info=mybir.SyncInfo(on_wait=[w], on_update=[]),
                        )
                        nop.engine = inst.engine
                        new_insts.append(nop)
                    si.on_wait = keep
                new_insts.append(inst)
            blk.instructions[:] = new_insts


def _strip_init_preamble(nc, init_names):
    """Drop the construction-time const-AP memsets and all-engine barrier:
    the const APs are unused here (bias comes from our own SBUF tensor)
    and every cross-engine edge in this program is explicitly sem-gated,
    so the barrier only serializes engine boot ahead of the DMA stream.
    Engine register preambles (RegisterMove) are kept."""
    drop_ops = {"Memset", "Drain", "EventSemaphore"}
    for fn in nc.m.functions:
        for blk in fn.blocks:
            blk.instructions[:] = [
                inst
                for inst in blk.instructions
                if not (inst.name in init_names and inst.opcode in drop_ops)
            ]


def build_nc_flat(scale, bias_val, tiles_spec=tuple(FLAT_TILES), strip_init=True):
    """Per-core SPMD program: y = tanh(scale*x + bias_val) over a flat
    stream tiled as [rows, cols] f32 tiles.

    All DMAs (ins AND outs) are issued from the SP HWDGE queue in
    program order: every in first, then each out as ACT's act_sem
    arrives.  Per-engine FIFO drain therefore gives the in-stream
    absolute priority -- the last in (which gates the last tanh)
    completes at half the stream time instead of at the end, and the
    out backlog then drains at full rate with no idle."""
    import contextlib

    import concourse.bass as bass
    from concourse import mybir

    scale = float(scale)
    bias_val = float(bias_val)
    T = len(tiles_spec)
    nc = bass.Bass()
    init_names = {
        inst.name for fn in nc.m.functions for blk in fn.blocks
        for inst in blk.instructions
    }
    xs = [
        nc.declare_dram_parameter(f"x{t}", [r, c], mybir.dt.float32, isOutput=False)
        for t, (r, c) in enumerate(tiles_spec)
    ]
    # Outputs leave the device as bf16: tanh's range is (-1, 1) and the
    # harness gate is rel_err < 2e-2, while bf16 rounding contributes
    # only ~2e-3 elementwise (~1e-3 Frobenius).  This halves the write
    # stream, which is otherwise half the HBM traffic; the host upcasts
    # back to f32 during unshard.
    ys = [
        nc.declare_dram_parameter(f"y{t}", [r, c], mybir.dt.bfloat16, isOutput=True)
        for t, (r, c) in enumerate(tiles_spec)
    ]
    offs = np.concatenate([[0], np.cumsum([c for _, c in tiles_spec])]).astype(int)
    with contextlib.ExitStack() as ctx:
        tiles = ctx.enter_context(
            nc.sbuf_tensor([PART, int(offs[-1])], mybir.dt.float32)
        )
        otiles = ctx.enter_context(
            nc.sbuf_tensor([PART, int(offs[-1])], mybir.dt.bfloat16)
        )
        cb = ctx.enter_context(nc.sbuf_tensor([PART, 1], mybir.dt.float32))
        in_sems = [ctx.enter_context(nc.semaphore(f"in_sem{t}")) for t in range(T)]
        act_sem = ctx.enter_context(nc.semaphore("act_sem"))
        out_sem = ctx.enter_context(nc.semaphore("out_sem"))
        cb_sem = ctx.enter_context(nc.semaphore("cb_sem"))
        block = ctx.enter_context(nc.Block())

        def tile_ap(t):
            r = tiles_spec[t][0]
            return tiles.ap()[:r, int(offs[t]) : int(offs[t + 1])]

        def otile_ap(t):
            r = tiles_spec[t][0]
            return otiles.ap()[:r, int(offs[t]) : int(offs[t + 1])]

        @block.gpsimd
        def _(gpsimd):
            # Bias column; gpsimd is otherwise idle and off the DMA ring.
            # Drain before signalling: the inc must mean "value is in
            # SBUF", not "memset retired".
            gpsimd.memset(cb.ap()[:, 0:1], bias_val)
            gpsimd.drain().then_inc(cb_sem, 1)

        @block.sync
        def _(sync):
            # Single strict-FIFO queue, ALL ins enqueued before any out.
            # Mixed read+write SDMA traffic tops out at ~346 GB/s while
            # pure reads reach ~378 and pure writes ~352, so a serial
            # read-phase/write-phase schedule is fastest; FIFO order also
            # hands the in-stream absolute priority, so the last tanh
            # (gated on the last in) fires at mid-stream, not at the end.
            for t in range(T):
                sync.dma_start(tile_ap(t), xs[t].ap()).then_inc(in_sems[t], 16)
            for t in range(T):
                sync.wait_ge(act_sem, t + 1)
                sync.dma_start(ys[t].ap(), otile_ap(t)).then_inc(out_sem, 16)
            sync.wait_ge(out_sem, 16 * T)

        @block.scalar
        def _(scalar):
            scalar.wait_ge(cb_sem, 1)
            for t in range(T):
                r = tiles_spec[t][0]
                scalar.wait_ge(in_sems[t], 16)
                scalar.activation(
                    otile_ap(t), tile_ap(t),
                    mybir.ActivationFunctionType.Tanh,
                    bias=cb.ap()[:r, 0:1], scale=scale,
                )
                scalar.drain().then_inc(act_sem, 1)

    if strip_init:
        _strip_init_preamble(nc, init_names)
    _split_multi_waits(nc)
    return nc


def shard_inputs_flat(img, tiles_spec=tuple(FLAT_TILES)):
    """[32,3,512,512] -> 8 per-core maps of flat [r, c] tiles."""
    sizes = [r * c for r, c in tiles_spec]
    offs = np.concatenate([[0], np.cumsum(sizes)]).astype(int)
    maps = []
    for c in range(N_CORES):
        flat = img[c * IMGS_PER_CORE : (c + 1) * IMGS_PER_CORE].reshape(-1)
        maps.append(
            {
                f"x{t}": flat[offs[t] : offs[t + 1]].reshape(tiles_spec[t])
                for t in range(len(tiles_spec))
            }
        )
    return maps


def unshard_outputs_flat(results, tiles_spec=tuple(FLAT_TILES)):
    return np.concatenate(
        [
            np.concatenate(
                [
                    np.asarray(r[f"y{t}"], dtype=np.float32).reshape(-1)
                    for t in range(len(tiles_spec))
                ]
            ).reshape(IMGS_PER_CORE, C, H, W)
            for r in results
        ],
        axis=0,
    )


def build_nc(scales, biases, bufs=BUFS, strip_init=True, split_tail=False):
    """Per-core SPMD program: y[p] = tanh(scales[p%3] * x[p] + biases[p%3])
    for 12 [128,2048] f32 planes.  (Per-channel fallback path.)"""
    import contextlib

    import concourse.bass as bass
    from concourse import mybir

    scales = [float(s) for s in scales]
    biases = [float(b) for b in biases]
    n = PLANES_PER_CORE
    nc = bass.Bass()
    init_names = {
        inst.name for fn in nc.m.functions for blk in fn.blocks
        for inst in blk.instructions
    }
    x = nc.declare_dram_parameter(
        "x", [n, PART, COLS], mybir.dt.float32, isOutput=False
    )
    y = nc.declare_dram_parameter(
        "y", [n, PART, COLS], mybir.dt.float32, isOutput=True
    )
    with contextlib.ExitStack() as ctx:
        tiles = ctx.enter_context(
            nc.sbuf_tensor([PART, COLS * bufs], mybir.dt.float32)
        )
        cb = ctx.enter_context(nc.sbuf_tensor([PART, C], mybir.dt.float32))
        in_sems = [ctx.enter_context(nc.semaphore(f"in_sem{s}")) for s in range(bufs)]
        act_sem = ctx.enter_context(nc.semaphore("act_sem"))
        out_sem = ctx.enter_context(nc.semaphore("out_sem"))
        cb_sem = ctx.enter_context(nc.semaphore("cb_sem"))
        block = ctx.enter_context(nc.Block())

        def tile_ap(p):
            return tiles.ap()[:, (p % bufs) * COLS : (p % bufs + 1) * COLS]

        @block.gpsimd
        def _(gpsimd):
            for c in range(C):
                gpsimd.memset(cb.ap()[:, c : c + 1], biases[c])
            gpsimd.drain().then_inc(cb_sem, 1)

        @block.sync
        def _(sync):
            for p in range(min(bufs, n)):
                sync.dma_start(tile_ap(p), x[p]).then_inc(in_sems[p % bufs], 16)
            for p in range(n):
                sync.wait_ge(act_sem, p + 1)
                sync.dma_start(y[p], tile_ap(p)).then_inc(out_sem, 16)
                if p + bufs < n:
                    sync.dma_start(tile_ap(p + bufs), x[p + bufs]).then_inc(
                        in_sems[(p + bufs) % bufs], 16
                    )
            sync.wait_ge(out_sem, 16 * n)

        @block.scalar
        def _(scalar):
            scalar.wait_ge(cb_sem, 1)
            for p in range(n):
                c = p % C
                scalar.wait_ge(in_sems[p % bufs], 16 * (p // bufs + 1))
                scalar.activation(
                    tile_ap(p), tile_ap(p),
                    mybir.ActivationFunctionType.Tanh,
                    bias=cb.ap()[:, c : c + 1], scale=scales[c],
                )
                scalar.drain().then_inc(act_sem, 1)

    if strip_init:
        _strip_init_preamble(nc, init_names)
    _split_multi_waits(nc)
    return nc


def shard_inputs(img):
    """[32,3,512,512] -> 8 per-core input maps of [12,128,2048]."""
    return [
        {
            "x": img[c * IMGS_PER_CORE : (c + 1) * IMGS_PER_CORE].reshape(
                PLANES_PER_CORE, PART, COLS
            )
        }
        for c in range(N_CORES)
    ]


def unshard_outputs(results):
    return np.concatenate(
        [r["y"].reshape(IMGS_PER_CORE, C, H, W) for r in results], axis=0
    )


def _general_host_path(img, weight, bias):
    """Bit-faithful numpy replica of the reference for arbitrary tables."""
    x = np.transpose(img, (0, 2, 3, 1))
    rgb = (x + np.float32(1.0)) * np.float32(127.5)
    idx = (
        rgb[..., 0] * np.float32(65536.0)
        + rgb[..., 1] * np.float32(256.0)
        + rgb[..., 2]
    ).astype(np.int32)
    y = np.tanh(weight[idx] * x + bias[idx])
    return np.ascontiguousarray(np.transpose(y, (0, 3, 1, 2)).astype(np.float32))


def plan(weight, bias):
    """Pick the device program for these tables.

    Returns (nc, shard_fn, unshard_fn) or None if the host fallback is
    required (LUT rows differ, so the per-pixel gather matters)."""
    weight = np.asarray(weight, dtype=np.float32)
    bias = np.asarray(bias, dtype=np.float32)
    rows_const = (
        (weight.min(axis=0) == weight.max(axis=0)).all()
        and (bias.min(axis=0) == bias.max(axis=0)).all()
    )
    if not rows_const:
        return None
    w0, b0 = weight[0], bias[0]
    if (w0 == w0[0]).all() and (b0 == b0[0]).all():
        return (
            build_nc_flat(w0[0], b0[0]),
            shard_inputs_flat,
            unshard_outputs_flat,
        )
    return build_nc(w0, b0), shard_inputs, unshard_outputs


def kernel(img, weight, bias):
    img = np.ascontiguousarray(np.asarray(img, dtype=np.float32))
    assert img.shape == (B, C, H, W), img.shape

    p = plan(weight, bias)
    if p is None:
        return _general_host_path(img, weight, bias)
    nc, shard_fn, unshard_fn = p

    from concourse.bass_utils import run_bass_kernel_spmd

    res = run_bass_kernel_spmd(nc, shard_fn(img), list(range(N_CORES)))
    return unshard_fn(res.results)


# revision 27
# speedup vs baseline: 1.0122x; 1.0122x over previous
"""Trainium2 kernel for nn_ColorMapGenerator.

Reference semantics (NCHW in / NCHW out):
    x   = img.transpose(0,2,3,1)                 # [B,H,W,3]
    rgb = (x + 1) * 127.5
    idx = (rgb[...,0]*65536 + rgb[...,1]*256 + rgb[...,2]).astype(int32)
    y   = tanh(weight[idx] * x + bias[idx])      # per-pixel LUT rows
    out = y.transpose(0,3,1,2)                   # [B,3,H,W]

The 16.7M-row weight/bias tables are checked on the host:

* every row identical AND the row itself uniform across channels (true
  for this problem's inputs: weight all ones, bias all zeros) -> the op
  is y = tanh(s*x + b) elementwise with scalar s, b.  Channel/layout
  structure is irrelevant, so each core streams its flat 12MB slice of
  the batch through SBUF in [128, c] tiles whose DRAM rows are up to
  16KB contiguous (bigger DMA packets than the 8KB plane rows).
* rows identical but per-channel values differ -> per-channel plane
  kernel (12 [128,2048] planes per core, scale immediate + bias column).
* rows differ -> host numpy fallback (the gather actually matters).

Flat-kernel design (per core, raw Bass), measured on hardware:
  - every tile has its own SBUF slot (no reuse; ~12MB in + ~6MB out
    resident) and its own in-DMA semaphore.
  - ALL DMAs issue from the single SP HWDGE queue in program order: all
    ins first, then each out as ACT's act_sem arrives.  Strict per-queue
    FIFO (a) gives the in-stream absolute priority so the last tanh
    (gated on the last in) fires mid-stream, and (b) keeps reads and
    writes in separate phases: pure reads sustain ~380-410 GB/s and
    pure writes ~350-390, while 1:1 mixed traffic drops to ~350 GB/s.
  - ACT gates each tanh on the tile's in-sem reaching 16 (all 16 SDMA
    engines done - a single cumulative semaphore would be unsound: a
    fast engine's increments for later DMAs can stand in for a lagging
    engine's), drains its datapath, then incs act_sem to release the
    out-DMA (then_inc alone fires at sequencer retire, not datapath
    completion).
  - outputs leave the device as bf16 (~2e-3 elementwise rounding vs the
    2e-2 gate), halving the write stream; the host upcasts in unshard.
  - SP parks on the final out_sem target so the program ends only when
    every out-DMA completed.
  - walrus in this toolchain encodes at most ONE sync-wait per
    instruction; _split_multi_waits hoists extras onto standalone NoOps.
"""

import numpy as np

B, C, H, W = 32, 3, 512, 512
N_CORES = 8
IMGS_PER_CORE = B // N_CORES           # 4
PLANES_PER_CORE = IMGS_PER_CORE * C    # 12 [128,2048] planes per core
PART = 128
COLS = (H * W) // PART                 # 2048
BUFS = 6

# Flat-path tile schedule: (rows, cols) per tile, sum(rows*cols) must
# equal the per-core element count.  Cols stay powers of two: 8KB/16KB
# packets run at the full ~26.5-27 GB/s per SDMA engine (odd sizes like
# 16320B run at half rate).  A DMA's rows are sprayed over the largest
# divisor of the row count <= 16 SDMA engines (allocated contiguously
# from engine 64): 128 rows -> all 16 engines x 8 rows, 120 rows -> 15
# engines x 8 rows with engine 79 skipped.  Engine 79 also hosts HWDGE
# queue management and intermittently runs 10-20% slower, so one
# 120-row tile trims its byte share to ~0.92 of the others', bounding
# the straggle without starving it (measured best across trim levels).
# The small head tile starts the ACT pipeline early; the tiny 16-row
# tile last keeps the write-only tail short.  More tiles were measured
# to cost ~0.5-1us of queue dispatch per extra DMA pair, so T=8 is the
# sweet spot.
FLAT_TILES = [
    (128, 2048),
    (128, 4096), (128, 4096), (128, 4096), (128, 4096), (128, 4096),
    (120, 2048),
    (16, 1024),
]
assert sum(r * c for r, c in FLAT_TILES) == IMGS_PER_CORE * C * H * W


def _split_multi_waits(nc, max_waits=1):
    from concourse import mybir

    for fn in nc.m.functions:
        for blk in fn.blocks:
            new_insts = []
            for inst in blk.instructions:
                si = inst.sync_info
                if si is not None and si.on_wait and len(si.on_wait) > max_waits:
                    waits = list(si.on_wait)
                    extra, keep = waits[:-max_waits], waits[-max_waits:]
                    for w in extra:
                        nop = mybir.InstNoOp(
                            name=nc.get_next_instruction_name(),
                            ins=[],
                            outs=[],
                            sync_info=mybir.SyncInfo(on_wait=[w], on_update=[]),
                        )
                        nop.engine = inst.engine
                        new_insts.append(nop)
                    si.on_wait = keep
                new_insts.append(inst)
            blk.instructions[:] = new_insts


def _strip_init_preamble(nc, init_names):
    """Drop the construction-time const-AP memsets and all-engine barrier:
    the const APs are unused here (bias comes from our own SBUF tensor)
    and every cross-engine edge in this program is explicitly sem-gated,
    so the barrier only serializes engine boot ahead of the DMA stream.
    Engine register preambles (RegisterMove) are kept."""
    drop_ops = {"Memset", "Drain", "EventSemaphore"}
    for fn in nc.m.functions:
        for blk in fn.blocks:
            blk.instructions[:] = [
                inst
                for inst in blk.instructions
                if not (inst.name in init_names and inst.opcode in drop_ops)
            ]


def build_nc_flat(scale, bias_val, tiles_spec=tuple(FLAT_TILES), strip_init=True):
    """Per-core SPMD program: y = tanh(scale*x + bias_val) over a flat
    stream tiled as [rows, cols] f32 tiles.

    All DMAs (ins AND outs) are issued from the SP HWDGE queue in
    program order: every in first, then each out as ACT's act_sem
    arrives.  Per-engine FIFO drain therefore gives the in-stream
    absolute priority -- the last in (which gates the last tanh)
    completes at half the stream time instead of at the end, and the
    out backlog then drains at full rate with no idle."""
    import contextlib

    import concourse.bass as bass
    from concourse import mybir

    scale = float(scale)
    bias_val = float(bias_val)
    T = len(tiles_spec)
    nc = bass.Bass()
    init_names = {
        inst.name for fn in nc.m.functions for blk in fn.blocks
        for inst in blk.instructions
    }
    xs = [
        nc.declare_dram_parameter(f"x{t}", [r, c], mybir.dt.float32, isOutput=False)
        for t, (r, c) in enumerate(tiles_spec)
    ]
    # Outputs leave the device as bf16: tanh's range is (-1, 1) and the
    # harness gate is rel_err < 2e-2, while bf16 rounding contributes
    # only ~2e-3 elementwise (~1e-3 Frobenius).  This halves the write
    # stream, which is otherwise half the HBM traffic; the host upcasts
    # back to f32 during unshard.
    ys = [
        nc.declare_dram_parameter(f"y{t}", [r, c], mybir.dt.bfloat16, isOutput=True)
        for t, (r, c) in enumerate(tiles_spec)
    ]
    offs = np.concatenate([[0], np.cumsum([c for _, c in tiles_spec])]).astype(int)
    with contextlib.ExitStack() as ctx:
        tiles = ctx.enter_context(
            nc.sbuf_tensor([PART, int(offs[-1])], mybir.dt.float32)
        )
        otiles = ctx.enter_context(
            nc.sbuf_tensor([PART, int(offs[-1])], mybir.dt.bfloat16)
        )
        cb = ctx.enter_context(nc.sbuf_tensor([PART, 1], mybir.dt.float32))
        in_sems = [ctx.enter_context(nc.semaphore(f"in_sem{t}")) for t in range(T)]
        act_sem = ctx.enter_context(nc.semaphore("act_sem"))
        out_sem = ctx.enter_context(nc.semaphore("out_sem"))
        cb_sem = ctx.enter_context(nc.semaphore("cb_sem"))
        block = ctx.enter_context(nc.Block())

        def tile_ap(t):
            r = tiles_spec[t][0]
            return tiles.ap()[:r, int(offs[t]) : int(offs[t + 1])]

        def otile_ap(t):
            r = tiles_spec[t][0]
            return otiles.ap()[:r, int(offs[t]) : int(offs[t + 1])]

        @block.gpsimd
        def _(gpsimd):
            # Bias column; gpsimd is otherwise idle and off the DMA ring.
            # Drain before signalling: the inc must mean "value is in
            # SBUF", not "memset retired".
            gpsimd.memset(cb.ap()[:, 0:1], bias_val)
            gpsimd.drain().then_inc(cb_sem, 1)

        @block.sync
        def _(sync):
            # Single strict-FIFO queue, ALL ins enqueued before any out.
            # Mixed read+write SDMA traffic tops out at ~346 GB/s while
            # pure reads reach ~378 and pure writes ~352, so a serial
            # read-phase/write-phase schedule is fastest; FIFO order also
            # hands the in-stream absolute priority, so the last tanh
            # (gated on the last in) fires at mid-stream, not at the end.
            for t in range(T):
                sync.dma_start(tile_ap(t), xs[t].ap()).then_inc(in_sems[t], 16)
            for t in range(T):
                sync.wait_ge(act_sem, t + 1)
                sync.dma_start(ys[t].ap(), otile_ap(t)).then_inc(out_sem, 16)
            sync.wait_ge(out_sem, 16 * T)

        @block.scalar
        def _(scalar):
            scalar.wait_ge(cb_sem, 1)
            for t in range(T):
                r = tiles_spec[t][0]
                scalar.wait_ge(in_sems[t], 16)
                scalar.activation(
                    otile_ap(t), tile_ap(t),
                    mybir.ActivationFunctionType.Tanh,
                    bias=cb.ap()[:r, 0:1], scale=scale,
                )
                scalar.drain().then_inc(act_sem, 1)

    if strip_init:
        _strip_init_preamble(nc, init_names)
    _split_multi_waits(nc)
    return nc


def shard_inputs_flat(img, tiles_spec=tuple(FLAT_TILES)):
    """[32,3,512,512] -> 8 per-core maps of flat [r, c] tiles."""
    sizes = [r * c for r, c in tiles_spec]
    offs = np.concatenate([[0], np.cumsum(sizes)]).astype(int)
    maps = []
    for c in range(N_CORES):
        flat = img[c * IMGS_PER_CORE : (c + 1) * IMGS_PER_CORE].reshape(-1)
        maps.append(
            {
                f"x{t}": flat[offs[t] : offs[t + 1]].reshape(tiles_spec[t])
                for t in range(len(tiles_spec))
            }
        )
    return maps


def unshard_outputs_flat(results, tiles_spec=tuple(FLAT_TILES)):
    return np.concatenate(
        [
            np.concatenate(
                [
                    np.asarray(r[f"y{t}"], dtype=np.float32).reshape(-1)
                    for t in range(len(tiles_spec))
                ]
            ).reshape(IMGS_PER_CORE, C, H, W)
            for r in results
        ],
        axis=0,
    )


def build_nc(scales, biases, bufs=BUFS, strip_init=True, split_tail=False):
    """Per-core SPMD program: y[p] = tanh(scales[p%3] * x[p] + biases[p%3])
    for 12 [128,2048] f32 planes.  (Per-channel fallback path.)"""
    import contextlib

    import concourse.bass as bass
    from concourse import mybir

    scales = [float(s) for s in scales]
    biases = [float(b) for b in biases]
    n = PLANES_PER_CORE
    nc = bass.Bass()
    init_names = {
        inst.name for fn in nc.m.functions for blk in fn.blocks
        for inst in blk.instructions
    }
    x = nc.declare_dram_parameter(
        "x", [n, PART, COLS], mybir.dt.float32, isOutput=False
    )
    y = nc.declare_dram_parameter(
        "y", [n, PART, COLS], mybir.dt.float32, isOutput=True
    )
    with contextlib.ExitStack() as ctx:
        tiles = ctx.enter_context(
            nc.sbuf_tensor([PART, COLS * bufs], mybir.dt.float32)
        )
        cb = ctx.enter_context(nc.sbuf_tensor([PART, C], mybir.dt.float32))
        in_sems = [ctx.enter_context(nc.semaphore(f"in_sem{s}")) for s in range(bufs)]
        act_sem = ctx.enter_context(nc.semaphore("act_sem"))
        out_sem = ctx.enter_context(nc.semaphore("out_sem"))
        cb_sem = ctx.enter_context(nc.semaphore("cb_sem"))
        block = ctx.enter_context(nc.Block())

        def tile_ap(p):
            return tiles.ap()[:, (p % bufs) * COLS : (p % bufs + 1) * COLS]

        @block.gpsimd
        def _(gpsimd):
            for c in range(C):
                gpsimd.memset(cb.ap()[:, c : c + 1], biases[c])
            gpsimd.drain().then_inc(cb_sem, 1)

        @block.sync
        def _(sync):
            for p in range(min(bufs, n)):
                sync.dma_start(tile_ap(p), x[p]).then_inc(in_sems[p % bufs], 16)
            for p in range(n):
                sync.wait_ge(act_sem, p + 1)
                sync.dma_start(y[p], tile_ap(p)).then_inc(out_sem, 16)
                if p + bufs < n:
                    sync.dma_start(tile_ap(p + bufs), x[p + bufs]).then_inc(
                        in_sems[(p + bufs) % bufs], 16
                    )
            sync.wait_ge(out_sem, 16 * n)

        @block.scalar
        def _(scalar):
            scalar.wait_ge(cb_sem, 1)
            for p in range(n):
                c = p % C
                scalar.wait_ge(in_sems[p % bufs], 16 * (p // bufs + 1))
                scalar.activation(
                    tile_ap(p), tile_ap(p),
                    mybir.ActivationFunctionType.Tanh,
                    bias=cb.ap()[:, c : c + 1], scale=scales[c],
                )
                scalar.drain().then_inc(act_sem, 1)

    if strip_init:
        _strip_init_preamble(nc, init_names)
    _split_multi_waits(nc)
    return nc


def shard_inputs(img):
    """[32,3,512,512] -> 8 per-core input maps of [12,128,2048]."""
    return [
        {
            "x": img[c * IMGS_PER_CORE : (c + 1) * IMGS_PER_CORE].reshape(
                PLANES_PER_CORE, PART, COLS
            )
        }
        for c in range(N_CORES)
    ]


def unshard_outputs(results):
    return np.concatenate(
        [r["y"].reshape(IMGS_PER_CORE, C, H, W) for r in results], axis=0
    )


def _general_host_path(img, weight, bias):
    """Bit-faithful numpy replica of the reference for arbitrary tables."""
    x = np.transpose(img, (0, 2, 3, 1))
    rgb = (x + np.float32(1.0)) * np.float32(127.5)
    idx = (
        rgb[..., 0] * np.float32(65536.0)
        + rgb[..., 1] * np.float32(256.0)
        + rgb[..., 2]
    ).astype(np.int32)
    y = np.tanh(weight[idx] * x + bias[idx])
    return np.ascontiguousarray(np.transpose(y, (0, 3, 1, 2)).astype(np.float32))


def plan(weight, bias):
    """Pick the device program for these tables.

    Returns (nc, shard_fn, unshard_fn) or None if the host fallback is
    required (LUT rows differ, so the per-pixel gather matters)."""
    weight = np.asarray(weight, dtype=np.float32)
    bias = np.asarray(bias, dtype=np.float32)
    rows_const = (
        (weight.min(axis=0) == weight.max(axis=0)).all()
        and (bias.min(axis=0) == bias.max(axis=0)).all()
    )
    if not rows_const:
        return None
    w0, b0 = weight[0], bias[0]
    if (w0 == w0[0]).all() and (b0 == b0[0]).all():
        return (
            build_nc_flat(w0[0], b0[0]),
            shard_inputs_flat,
            unshard_outputs_flat,
        )
    return build_nc(w0, b0), shard_inputs, unshard_outputs


def kernel(img, weight, bias):
    img = np.ascontiguousarray(np.asarray(img, dtype=np.float32))
    assert img.shape == (B, C, H, W), img.shape

    p = plan(weight, bias)
    if p is None:
        return _general_host_path(img, weight, bias)
    nc, shard_fn, unshard_fn = p

    from concourse.bass_utils import run_bass_kernel_spmd

    res = run_bass_kernel_spmd(nc, shard_fn(img), list(range(N_CORES)))
    return unshard_fn(res.results)


# revision 30
# speedup vs baseline: 1.0270x; 1.0147x over previous
"""Trainium2 kernel for nn_ColorMapGenerator.

Reference semantics (NCHW in / NCHW out):
    x   = img.transpose(0,2,3,1)                 # [B,H,W,3]
    rgb = (x + 1) * 127.5
    idx = (rgb[...,0]*65536 + rgb[...,1]*256 + rgb[...,2]).astype(int32)
    y   = tanh(weight[idx] * x + bias[idx])      # per-pixel LUT rows
    out = y.transpose(0,3,1,2)                   # [B,3,H,W]

The 16.7M-row weight/bias tables are checked on the host:

* every row identical AND the row itself uniform across channels (true
  for this problem's inputs: weight all ones, bias all zeros) -> the op
  is y = tanh(s*x + b) elementwise with scalar s, b.  Channel/layout
  structure is irrelevant, so each core streams its flat 12MB slice of
  the batch through SBUF in [128, c] tiles whose DRAM rows are up to
  16KB contiguous (bigger DMA packets than the 8KB plane rows).
* rows identical but per-channel values differ -> per-channel plane
  kernel (12 [128,2048] planes per core, scale immediate + bias column).
* rows differ -> host numpy fallback (the gather actually matters).

Flat-kernel design (per core, raw Bass), measured on hardware:
  - every tile has its own SBUF slot (no reuse; ~12MB in + ~6MB out
    resident) and its own in-DMA semaphore.
  - ALL DMAs issue from the single SP HWDGE queue in program order: all
    ins first, then each out as ACT's act_sem arrives.  Strict per-queue
    FIFO (a) gives the in-stream absolute priority so the last tanh
    (gated on the last in) fires mid-stream, and (b) keeps reads and
    writes in separate phases: pure reads sustain ~380-410 GB/s and
    pure writes ~350-390, while 1:1 mixed traffic drops to ~350 GB/s.
  - ACT gates each tanh on the tile's in-sem reaching 16 (all 16 SDMA
    engines done - a single cumulative semaphore would be unsound: a
    fast engine's increments for later DMAs can stand in for a lagging
    engine's), drains its datapath, then incs act_sem to release the
    out-DMA (then_inc alone fires at sequencer retire, not datapath
    completion).
  - outputs leave the device as bf16 (~2e-3 elementwise rounding vs the
    2e-2 gate), halving the write stream; the host upcasts in unshard.
  - SP parks on the final out_sem target so the program ends only when
    every out-DMA completed.
  - walrus in this toolchain encodes at most ONE sync-wait per
    instruction; _split_multi_waits hoists extras onto standalone NoOps.
"""

import numpy as np

B, C, H, W = 32, 3, 512, 512
N_CORES = 8
IMGS_PER_CORE = B // N_CORES           # 4
PLANES_PER_CORE = IMGS_PER_CORE * C    # 12 [128,2048] planes per core
PART = 128
COLS = (H * W) // PART                 # 2048
BUFS = 6

# Flat-path tile schedule: (rows, cols) per tile, sum(rows*cols) must
# equal the per-core element count.  Cols stay powers of two: 8KB/16KB
# packets run at the full ~26.5-27 GB/s per SDMA engine (odd sizes like
# 16320B run at half rate).  A DMA's rows are sprayed over the largest
# divisor of the row count <= 16 SDMA engines (allocated contiguously
# from engine 64): 128 rows -> all 16 engines x 8 rows, 120 rows -> 15
# engines x 8 rows with engine 79 skipped.  Engine 79 also hosts HWDGE
# queue management and intermittently runs 10-20% slower, so one
# 120-row tile trims its byte share to ~0.92 of the others', bounding
# the straggle without starving it (measured best across trim levels).
# The small head tile starts the ACT pipeline early; the tiny 16-row
# tile last keeps the write-only tail short.  More tiles were measured
# to cost ~0.5-1us of queue dispatch per extra DMA pair, so T=8 is the
# sweet spot.
FLAT_TILES = [
    (128, 2048),
    (128, 4096), (128, 4096), (128, 4096), (128, 4096), (128, 4096),
    (120, 2048),
    (16, 1024),
]
assert sum(r * c for r, c in FLAT_TILES) == IMGS_PER_CORE * C * H * W


def _split_multi_waits(nc, max_waits=1):
    from concourse import mybir

    for fn in nc.m.functions:
        for blk in fn.blocks:
            new_insts = []
            for inst in blk.instructions:
                si = inst.sync_info
                if si is not None and si.on_wait and len(si.on_wait) > max_waits:
                    waits = list(si.on_wait)
                    extra, keep = waits[:-max_waits], waits[-max_waits:]
                    for w in extra:
                        nop = mybir.InstNoOp(
                            name=nc.get_next_instruction_name(),
                            ins=[],
                            outs=[],
                            sync_info=mybir.SyncInfo(on_wait=[w], on_update=[]),
                        )
                        nop.engine = inst.engine
                        new_insts.append(nop)
                    si.on_wait = keep
                new_insts.append(inst)
            blk.instructions[:] = new_insts


def _strip_init_preamble(nc, init_names):
    """Drop the construction-time const-AP memsets and all-engine barrier:
    the const APs are unused here (bias comes from our own SBUF tensor)
    and every cross-engine edge in this program is explicitly sem-gated,
    so the barrier only serializes engine boot ahead of the DMA stream.
    Engine register preambles (RegisterMove) are kept."""
    drop_ops = {"Memset", "Drain", "EventSemaphore"}
    for fn in nc.m.functions:
        for blk in fn.blocks:
            blk.instructions[:] = [
                inst
                for inst in blk.instructions
                if not (inst.name in init_names and inst.opcode in drop_ops)
            ]


def build_nc_flat(scale, bias_val, tiles_spec=tuple(FLAT_TILES), strip_init=True,
                  strip_teardown=False):
    """Per-core SPMD program: y = tanh(scale*x + bias_val) over a flat
    stream tiled as [rows, cols] f32 tiles.

    All DMAs (ins AND outs) are issued from the SP HWDGE queue in
    program order: every in first, then each out as ACT's act_sem
    arrives.  Per-engine FIFO drain therefore gives the in-stream
    absolute priority -- the last in (which gates the last tanh)
    completes at half the stream time instead of at the end, and the
    out backlog then drains at full rate with no idle."""
    import contextlib

    import concourse.bass as bass
    from concourse import mybir

    scale = float(scale)
    bias_val = float(bias_val)
    T = len(tiles_spec)
    nc = bass.Bass()
    init_names = {
        inst.name for fn in nc.m.functions for blk in fn.blocks
        for inst in blk.instructions
    }
    xs = [
        nc.declare_dram_parameter(f"x{t}", [r, c], mybir.dt.float32, isOutput=False)
        for t, (r, c) in enumerate(tiles_spec)
    ]
    # Outputs leave the device as bf16: tanh's range is (-1, 1) and the
    # harness gate is rel_err < 2e-2, while bf16 rounding contributes
    # only ~2e-3 elementwise (~1e-3 Frobenius).  This halves the write
    # stream, which is otherwise half the HBM traffic; the host upcasts
    # back to f32 during unshard.
    ys = [
        nc.declare_dram_parameter(f"y{t}", [r, c], mybir.dt.bfloat16, isOutput=True)
        for t, (r, c) in enumerate(tiles_spec)
    ]
    offs = np.concatenate([[0], np.cumsum([c for _, c in tiles_spec])]).astype(int)
    with contextlib.ExitStack() as ctx:
        tiles = ctx.enter_context(
            nc.sbuf_tensor([PART, int(offs[-1])], mybir.dt.float32)
        )
        otiles = ctx.enter_context(
            nc.sbuf_tensor([PART, int(offs[-1])], mybir.dt.bfloat16)
        )
        cb = ctx.enter_context(nc.sbuf_tensor([PART, 1], mybir.dt.float32))
        in_sems = [ctx.enter_context(nc.semaphore(f"in_sem{t}")) for t in range(T)]
        act_sem = ctx.enter_context(nc.semaphore("act_sem"))
        out_sem = ctx.enter_context(nc.semaphore("out_sem"))
        cb_sem = ctx.enter_context(nc.semaphore("cb_sem"))
        block = ctx.enter_context(nc.Block())

        def tile_ap(t):
            r = tiles_spec[t][0]
            return tiles.ap()[:r, int(offs[t]) : int(offs[t + 1])]

        def otile_ap(t):
            r = tiles_spec[t][0]
            return otiles.ap()[:r, int(offs[t]) : int(offs[t + 1])]

        @block.gpsimd
        def _(gpsimd):
            # Bias column; gpsimd is otherwise idle and off the DMA ring.
            # Drain before signalling: the inc must mean "value is in
            # SBUF", not "memset retired".
            gpsimd.memset(cb.ap()[:, 0:1], bias_val)
            gpsimd.drain().then_inc(cb_sem, 1)

        @block.sync
        def _(sync):
            # Single strict-FIFO queue, ALL ins enqueued before any out.
            # Mixed read+write SDMA traffic tops out at ~346 GB/s while
            # pure reads reach ~378 and pure writes ~352, so a serial
            # read-phase/write-phase schedule is fastest; FIFO order also
            # hands the in-stream absolute priority, so the last tanh
            # (gated on the last in) fires at mid-stream, not at the end.
            for t in range(T):
                sync.dma_start(tile_ap(t), xs[t].ap()).then_inc(in_sems[t], 16)
            for t in range(T):
                sync.wait_ge(act_sem, t + 1)
                sync.dma_start(ys[t].ap(), otile_ap(t)).then_inc(out_sem, 16)
            sync.wait_ge(out_sem, 16 * T)

        @block.scalar
        def _(scalar):
            scalar.wait_ge(cb_sem, 1)
            for t in range(T):
                r = tiles_spec[t][0]
                scalar.wait_ge(in_sems[t], 16)
                scalar.activation(
                    otile_ap(t), tile_ap(t),
                    mybir.ActivationFunctionType.Tanh,
                    bias=cb.ap()[:r, 0:1], scale=scale,
                )
                scalar.drain().then_inc(act_sem, 1)

    if strip_init:
        _strip_init_preamble(nc, init_names)
    if strip_teardown:
        _strip_exit_barrier(nc)
    _split_multi_waits(nc)
    return nc


def _strip_exit_barrier(nc):
    """Drop the Block-exit all-engine barrier (Drain + EventSemaphore
    rendezvous in the final block).  SP's out_sem wait already proves
    every out-DMA completed before SP retires; the other engines have
    no post-stream effects to order."""
    for fn in nc.m.functions:
        if not fn.blocks:
            continue
        blk = fn.blocks[-1]
        blk.instructions[:] = [
            inst
            for inst in blk.instructions
            if inst.opcode not in ("Drain", "EventSemaphore")
        ]


def shard_inputs_flat(img, tiles_spec=tuple(FLAT_TILES)):
    """[32,3,512,512] -> 8 per-core maps of flat [r, c] tiles."""
    sizes = [r * c for r, c in tiles_spec]
    offs = np.concatenate([[0], np.cumsum(sizes)]).astype(int)
    maps = []
    for c in range(N_CORES):
        flat = img[c * IMGS_PER_CORE : (c + 1) * IMGS_PER_CORE].reshape(-1)
        maps.append(
            {
                f"x{t}": flat[offs[t] : offs[t + 1]].reshape(tiles_spec[t])
                for t in range(len(tiles_spec))
            }
        )
    return maps


def unshard_outputs_flat(results, tiles_spec=tuple(FLAT_TILES)):
    return np.concatenate(
        [
            np.concatenate(
                [
                    np.asarray(r[f"y{t}"], dtype=np.float32).reshape(-1)
                    for t in range(len(tiles_spec))
                ]
            ).reshape(IMGS_PER_CORE, C, H, W)
            for r in results
        ],
        axis=0,
    )


def build_nc(scales, biases, bufs=BUFS, strip_init=True, split_tail=False):
    """Per-core SPMD program: y[p] = tanh(scales[p%3] * x[p] + biases[p%3])
    for 12 [128,2048] f32 planes.  (Per-channel fallback path.)"""
    import contextlib

    import concourse.bass as bass
    from concourse import mybir

    scales = [float(s) for s in scales]
    biases = [float(b) for b in biases]
    n = PLANES_PER_CORE
    nc = bass.Bass()
    init_names = {
        inst.name for fn in nc.m.functions for blk in fn.blocks
        for inst in blk.instructions
    }
    x = nc.declare_dram_parameter(
        "x", [n, PART, COLS], mybir.dt.float32, isOutput=False
    )
    y = nc.declare_dram_parameter(
        "y", [n, PART, COLS], mybir.dt.float32, isOutput=True
    )
    with contextlib.ExitStack() as ctx:
        tiles = ctx.enter_context(
            nc.sbuf_tensor([PART, COLS * bufs], mybir.dt.float32)
        )
        cb = ctx.enter_context(nc.sbuf_tensor([PART, C], mybir.dt.float32))
        in_sems = [ctx.enter_context(nc.semaphore(f"in_sem{s}")) for s in range(bufs)]
        act_sem = ctx.enter_context(nc.semaphore("act_sem"))
        out_sem = ctx.enter_context(nc.semaphore("out_sem"))
        cb_sem = ctx.enter_context(nc.semaphore("cb_sem"))
        block = ctx.enter_context(nc.Block())

        def tile_ap(p):
            return tiles.ap()[:, (p % bufs) * COLS : (p % bufs + 1) * COLS]

        @block.gpsimd
        def _(gpsimd):
            for c in range(C):
                gpsimd.memset(cb.ap()[:, c : c + 1], biases[c])
            gpsimd.drain().then_inc(cb_sem, 1)

        @block.sync
        def _(sync):
            for p in range(min(bufs, n)):
                sync.dma_start(tile_ap(p), x[p]).then_inc(in_sems[p % bufs], 16)
            for p in range(n):
                sync.wait_ge(act_sem, p + 1)
                sync.dma_start(y[p], tile_ap(p)).then_inc(out_sem, 16)
                if p + bufs < n:
                    sync.dma_start(tile_ap(p + bufs), x[p + bufs]).then_inc(
                        in_sems[(p + bufs) % bufs], 16
                    )
            sync.wait_ge(out_sem, 16 * n)

        @block.scalar
        def _(scalar):
            scalar.wait_ge(cb_sem, 1)
            for p in range(n):
                c = p % C
                scalar.wait_ge(in_sems[p % bufs], 16 * (p // bufs + 1))
                scalar.activation(
                    tile_ap(p), tile_ap(p),
                    mybir.ActivationFunctionType.Tanh,
                    bias=cb.ap()[:, c : c + 1], scale=scales[c],
                )
                scalar.drain().then_inc(act_sem, 1)

    if strip_init:
        _strip_init_preamble(nc, init_names)
    _split_multi_waits(nc)
    return nc


def shard_inputs(img):
    """[32,3,512,512] -> 8 per-core input maps of [12,128,2048]."""
    return [
        {
            "x": img[c * IMGS_PER_CORE : (c + 1) * IMGS_PER_CORE].reshape(
                PLANES_PER_CORE, PART, COLS
            )
        }
        for c in range(N_CORES)
    ]


def unshard_outputs(results):
    return np.concatenate(
        [r["y"].reshape(IMGS_PER_CORE, C, H, W) for r in results], axis=0
    )


def _general_host_path(img, weight, bias):
    """Bit-faithful numpy replica of the reference for arbitrary tables."""
    x = np.transpose(img, (0, 2, 3, 1))
    rgb = (x + np.float32(1.0)) * np.float32(127.5)
    idx = (
        rgb[..., 0] * np.float32(65536.0)
        + rgb[..., 1] * np.float32(256.0)
        + rgb[..., 2]
    ).astype(np.int32)
    y = np.tanh(weight[idx] * x + bias[idx])
    return np.ascontiguousarray(np.transpose(y, (0, 3, 1, 2)).astype(np.float32))


def plan(weight, bias):
    """Pick the device program for these tables.

    Returns (nc, shard_fn, unshard_fn) or None if the host fallback is
    required (LUT rows differ, so the per-pixel gather matters)."""
    weight = np.asarray(weight, dtype=np.float32)
    bias = np.asarray(bias, dtype=np.float32)
    rows_const = (
        (weight.min(axis=0) == weight.max(axis=0)).all()
        and (bias.min(axis=0) == bias.max(axis=0)).all()
    )
    if not rows_const:
        return None
    w0, b0 = weight[0], bias[0]
    if (w0 == w0[0]).all() and (b0 == b0[0]).all():
        return (
            build_nc_flat(w0[0], b0[0]),
            shard_inputs_flat,
            unshard_outputs_flat,
        )
    return build_nc(w0, b0), shard_inputs, unshard_outputs


def kernel(img, weight, bias):
    img = np.ascontiguousarray(np.asarray(img, dtype=np.float32))
    assert img.shape == (B, C, H, W), img.shape

    p = plan(weight, bias)
    if p is None:
        return _general_host_path(img, weight, bias)
    nc, shard_fn, unshard_fn = p

    from concourse.bass_utils import run_bass_kernel_spmd

    res = run_bass_kernel_spmd(nc, shard_fn(img), list(range(N_CORES)))
    return unshard_fn(res.results)


# revision 31
# speedup vs baseline: 1.1214x; 1.0919x over previous
"""Trainium2 kernel for nn_ColorMapGenerator.

Reference semantics (NCHW in / NCHW out):
    x   = img.transpose(0,2,3,1)                 # [B,H,W,3]
    rgb = (x + 1) * 127.5
    idx = (rgb[...,0]*65536 + rgb[...,1]*256 + rgb[...,2]).astype(int32)
    y   = tanh(weight[idx] * x + bias[idx])      # per-pixel LUT rows
    out = y.transpose(0,3,1,2)                   # [B,3,H,W]

The 16.7M-row weight/bias tables are checked on the host:

* every row identical AND the row itself uniform across channels (true
  for this problem's inputs: weight all ones, bias all zeros) -> the op
  is y = tanh(s*x + b) elementwise with scalar s, b.  Channel/layout
  structure is irrelevant, so each core streams its flat 12MB slice of
  the batch through SBUF in [128, c] tiles whose DRAM rows are up to
  16KB contiguous (bigger DMA packets than the 8KB plane rows).
* rows identical but per-channel values differ -> per-channel plane
  kernel (12 [128,2048] planes per core, scale immediate + bias column).
* rows differ -> host numpy fallback (the gather actually matters).

Flat-kernel design (per core, raw Bass), measured on hardware:
  - every tile has its own SBUF slot (no reuse; ~12MB in + ~6MB out
    resident) and its own in-DMA semaphore.
  - ALL DMAs issue from the single SP HWDGE queue in program order: all
    ins first, then each out as ACT's act_sem arrives.  Strict per-queue
    FIFO (a) gives the in-stream absolute priority so the last tanh
    (gated on the last in) fires mid-stream, and (b) keeps reads and
    writes in separate phases: pure reads sustain ~380-410 GB/s and
    pure writes ~350-390, while 1:1 mixed traffic drops to ~350 GB/s.
  - ACT gates each tanh on the tile's in-sem reaching 16 (all 16 SDMA
    engines done - a single cumulative semaphore would be unsound: a
    fast engine's increments for later DMAs can stand in for a lagging
    engine's), drains its datapath, then incs act_sem to release the
    out-DMA (then_inc alone fires at sequencer retire, not datapath
    completion).
  - outputs leave the device as bf16 (~2e-3 elementwise rounding vs the
    2e-2 gate), halving the write stream; the host upcasts in unshard.
  - SP parks on the final out_sem target so the program ends only when
    every out-DMA completed.
  - walrus in this toolchain encodes at most ONE sync-wait per
    instruction; _split_multi_waits hoists extras onto standalone NoOps.
"""

import numpy as np

B, C, H, W = 32, 3, 512, 512
N_CORES = 8
IMGS_PER_CORE = B // N_CORES           # 4
PLANES_PER_CORE = IMGS_PER_CORE * C    # 12 [128,2048] planes per core
PART = 128
COLS = (H * W) // PART                 # 2048
BUFS = 6

# Flat-path tile schedule: (rows, cols) per tile, sum(rows*cols) must
# equal the per-core element count.  Cols stay powers of two: 8KB/16KB
# packets run at the full ~26.5-27 GB/s per SDMA engine (odd sizes like
# 16320B run at half rate).  A DMA's rows are sprayed over the largest
# divisor of the row count <= 16 SDMA engines (allocated contiguously
# from engine 64): 128 rows -> all 16 engines x 8 rows, 120 rows -> 15
# engines x 8 rows with engine 79 skipped.  Engine 79 also hosts HWDGE
# queue management and intermittently runs 10-20% slower, so the
# 120-row tile trims its byte share to ~0.955 of the others' (measured
# best across trim levels 0.83/0.92/0.955/1.0: lighter trim wins clean
# runs, the slow mode is environmental and hits all levels).  Trim
# tiles sit late in the in-order and stay SMALL: a trimmed engine skips
# them and enters its write phase early, and mixed read/write traffic
# drops to ~350 GB/s (a 3.75MB trim tile cost 10us this way).  The
# small head tile starts the ACT pipeline early; the tiny tail tile
# keeps the write-only tail short.
FLAT_TILES = [
    (128, 2048),
    (128, 4096), (128, 4096), (128, 4096), (128, 4096), (128, 4096),
    (128, 1024),
    (120, 1024),
    (8, 1024),
]
assert sum(r * c for r, c in FLAT_TILES) == IMGS_PER_CORE * C * H * W


def _split_multi_waits(nc, max_waits=1):
    from concourse import mybir

    for fn in nc.m.functions:
        for blk in fn.blocks:
            new_insts = []
            for inst in blk.instructions:
                si = inst.sync_info
                if si is not None and si.on_wait and len(si.on_wait) > max_waits:
                    waits = list(si.on_wait)
                    extra, keep = waits[:-max_waits], waits[-max_waits:]
                    for w in extra:
                        nop = mybir.InstNoOp(
                            name=nc.get_next_instruction_name(),
                            ins=[],
                            outs=[],
                            sync_info=mybir.SyncInfo(on_wait=[w], on_update=[]),
                        )
                        nop.engine = inst.engine
                        new_insts.append(nop)
                    si.on_wait = keep
                new_insts.append(inst)
            blk.instructions[:] = new_insts


def _strip_init_preamble(nc, init_names):
    """Drop the construction-time const-AP memsets and all-engine barrier:
    the const APs are unused here (bias comes from our own SBUF tensor)
    and every cross-engine edge in this program is explicitly sem-gated,
    so the barrier only serializes engine boot ahead of the DMA stream.
    Engine register preambles (RegisterMove) are kept."""
    drop_ops = {"Memset", "Drain", "EventSemaphore"}
    for fn in nc.m.functions:
        for blk in fn.blocks:
            blk.instructions[:] = [
                inst
                for inst in blk.instructions
                if not (inst.name in init_names and inst.opcode in drop_ops)
            ]


def build_nc_flat(scale, bias_val, tiles_spec=tuple(FLAT_TILES), strip_init=True,
                  strip_teardown=False):
    """Per-core SPMD program: y = tanh(scale*x + bias_val) over a flat
    stream tiled as [rows, cols] f32 tiles.

    All DMAs (ins AND outs) are issued from the SP HWDGE queue in
    program order: every in first, then each out as ACT's act_sem
    arrives.  Per-engine FIFO drain therefore gives the in-stream
    absolute priority -- the last in (which gates the last tanh)
    completes at half the stream time instead of at the end, and the
    out backlog then drains at full rate with no idle."""
    import contextlib

    import concourse.bass as bass
    from concourse import mybir

    scale = float(scale)
    bias_val = float(bias_val)
    T = len(tiles_spec)
    nc = bass.Bass()
    init_names = {
        inst.name for fn in nc.m.functions for blk in fn.blocks
        for inst in blk.instructions
    }
    xs = [
        nc.declare_dram_parameter(f"x{t}", [r, c], mybir.dt.float32, isOutput=False)
        for t, (r, c) in enumerate(tiles_spec)
    ]
    # Outputs leave the device as bf16: tanh's range is (-1, 1) and the
    # harness gate is rel_err < 2e-2, while bf16 rounding contributes
    # only ~2e-3 elementwise (~1e-3 Frobenius).  This halves the write
    # stream, which is otherwise half the HBM traffic; the host upcasts
    # back to f32 during unshard.
    ys = [
        nc.declare_dram_parameter(f"y{t}", [r, c], mybir.dt.bfloat16, isOutput=True)
        for t, (r, c) in enumerate(tiles_spec)
    ]
    offs = np.concatenate([[0], np.cumsum([c for _, c in tiles_spec])]).astype(int)
    with contextlib.ExitStack() as ctx:
        tiles = ctx.enter_context(
            nc.sbuf_tensor([PART, int(offs[-1])], mybir.dt.float32)
        )
        otiles = ctx.enter_context(
            nc.sbuf_tensor([PART, int(offs[-1])], mybir.dt.bfloat16)
        )
        cb = ctx.enter_context(nc.sbuf_tensor([PART, 1], mybir.dt.float32))
        in_sems = [ctx.enter_context(nc.semaphore(f"in_sem{t}")) for t in range(T)]
        act_sem = ctx.enter_context(nc.semaphore("act_sem"))
        out_sem = ctx.enter_context(nc.semaphore("out_sem"))
        cb_sem = ctx.enter_context(nc.semaphore("cb_sem"))
        block = ctx.enter_context(nc.Block())

        def tile_ap(t):
            r = tiles_spec[t][0]
            return tiles.ap()[:r, int(offs[t]) : int(offs[t + 1])]

        def otile_ap(t):
            r = tiles_spec[t][0]
            return otiles.ap()[:r, int(offs[t]) : int(offs[t + 1])]

        @block.gpsimd
        def _(gpsimd):
            # Bias column; gpsimd is otherwise idle and off the DMA ring.
            # Drain before signalling: the inc must mean "value is in
            # SBUF", not "memset retired".
            gpsimd.memset(cb.ap()[:, 0:1], bias_val)
            gpsimd.drain().then_inc(cb_sem, 1)

        @block.sync
        def _(sync):
            # Single strict-FIFO queue, ALL ins enqueued before any out.
            # Mixed read+write SDMA traffic tops out at ~346 GB/s while
            # pure reads reach ~378 and pure writes ~352, so a serial
            # read-phase/write-phase schedule is fastest; FIFO order also
            # hands the in-stream absolute priority, so the last tanh
            # (gated on the last in) fires at mid-stream, not at the end.
            for t in range(T):
                sync.dma_start(tile_ap(t), xs[t].ap()).then_inc(in_sems[t], 16)
            for t in range(T):
                sync.wait_ge(act_sem, t + 1)
                sync.dma_start(ys[t].ap(), otile_ap(t)).then_inc(out_sem, 16)
            sync.wait_ge(out_sem, 16 * T)

        @block.scalar
        def _(scalar):
            scalar.wait_ge(cb_sem, 1)
            for t in range(T):
                r = tiles_spec[t][0]
                scalar.wait_ge(in_sems[t], 16)
                scalar.activation(
                    otile_ap(t), tile_ap(t),
                    mybir.ActivationFunctionType.Tanh,
                    bias=cb.ap()[:r, 0:1], scale=scale,
                )
                scalar.drain().then_inc(act_sem, 1)

    if strip_init:
        _strip_init_preamble(nc, init_names)
    if strip_teardown:
        _strip_exit_barrier(nc)
    _split_multi_waits(nc)
    return nc


def _strip_exit_barrier(nc):
    """Drop the Block-exit all-engine barrier (Drain + EventSemaphore
    rendezvous in the final block).  SP's out_sem wait already proves
    every out-DMA completed before SP retires; the other engines have
    no post-stream effects to order."""
    for fn in nc.m.functions:
        if not fn.blocks:
            continue
        blk = fn.blocks[-1]
        blk.instructions[:] = [
            inst
            for inst in blk.instructions
            if inst.opcode not in ("Drain", "EventSemaphore")
        ]


def shard_inputs_flat(img, tiles_spec=tuple(FLAT_TILES)):
    """[32,3,512,512] -> 8 per-core maps of flat [r, c] tiles."""
    sizes = [r * c for r, c in tiles_spec]
    offs = np.concatenate([[0], np.cumsum(sizes)]).astype(int)
    maps = []
    for c in range(N_CORES):
        flat = img[c * IMGS_PER_CORE : (c + 1) * IMGS_PER_CORE].reshape(-1)
        maps.append(
            {
                f"x{t}": flat[offs[t] : offs[t + 1]].reshape(tiles_spec[t])
                for t in range(len(tiles_spec))
            }
        )
    return maps


def unshard_outputs_flat(results, tiles_spec=tuple(FLAT_TILES)):
    return np.concatenate(
        [
            np.concatenate(
                [
                    np.asarray(r[f"y{t}"], dtype=np.float32).reshape(-1)
                    for t in range(len(tiles_spec))
                ]
            ).reshape(IMGS_PER_CORE, C, H, W)
            for r in results
        ],
        axis=0,
    )


def build_nc(scales, biases, bufs=BUFS, strip_init=True, split_tail=False):
    """Per-core SPMD program: y[p] = tanh(scales[p%3] * x[p] + biases[p%3])
    for 12 [128,2048] f32 planes.  (Per-channel fallback path.)"""
    import contextlib

    import concourse.bass as bass
    from concourse import mybir

    scales = [float(s) for s in scales]
    biases = [float(b) for b in biases]
    n = PLANES_PER_CORE
    nc = bass.Bass()
    init_names = {
        inst.name for fn in nc.m.functions for blk in fn.blocks
        for inst in blk.instructions
    }
    x = nc.declare_dram_parameter(
        "x", [n, PART, COLS], mybir.dt.float32, isOutput=False
    )
    y = nc.declare_dram_parameter(
        "y", [n, PART, COLS], mybir.dt.float32, isOutput=True
    )
    with contextlib.ExitStack() as ctx:
        tiles = ctx.enter_context(
            nc.sbuf_tensor([PART, COLS * bufs], mybir.dt.float32)
        )
        cb = ctx.enter_context(nc.sbuf_tensor([PART, C], mybir.dt.float32))
        in_sems = [ctx.enter_context(nc.semaphore(f"in_sem{s}")) for s in range(bufs)]
        act_sem = ctx.enter_context(nc.semaphore("act_sem"))
        out_sem = ctx.enter_context(nc.semaphore("out_sem"))
        cb_sem = ctx.enter_context(nc.semaphore("cb_sem"))
        block = ctx.enter_context(nc.Block())

        def tile_ap(p):
            return tiles.ap()[:, (p % bufs) * COLS : (p % bufs + 1) * COLS]

        @block.gpsimd
        def _(gpsimd):
            for c in range(C):
                gpsimd.memset(cb.ap()[:, c : c + 1], biases[c])
            gpsimd.drain().then_inc(cb_sem, 1)

        @block.sync
        def _(sync):
            for p in range(min(bufs, n)):
                sync.dma_start(tile_ap(p), x[p]).then_inc(in_sems[p % bufs], 16)
            for p in range(n):
                sync.wait_ge(act_sem, p + 1)
                sync.dma_start(y[p], tile_ap(p)).then_inc(out_sem, 16)
                if p + bufs < n:
                    sync.dma_start(tile_ap(p + bufs), x[p + bufs]).then_inc(
                        in_sems[(p + bufs) % bufs], 16
                    )
            sync.wait_ge(out_sem, 16 * n)

        @block.scalar
        def _(scalar):
            scalar.wait_ge(cb_sem, 1)
            for p in range(n):
                c = p % C
                scalar.wait_ge(in_sems[p % bufs], 16 * (p // bufs + 1))
                scalar.activation(
                    tile_ap(p), tile_ap(p),
                    mybir.ActivationFunctionType.Tanh,
                    bias=cb.ap()[:, c : c + 1], scale=scales[c],
                )
                scalar.drain().then_inc(act_sem, 1)

    if strip_init:
        _strip_init_preamble(nc, init_names)
    _split_multi_waits(nc)
    return nc


def shard_inputs(img):
    """[32,3,512,512] -> 8 per-core input maps of [12,128,2048]."""
    return [
        {
            "x": img[c * IMGS_PER_CORE : (c + 1) * IMGS_PER_CORE].reshape(
                PLANES_PER_CORE, PART, COLS
            )
        }
        for c in range(N_CORES)
    ]


def unshard_outputs(results):
    return np.concatenate(
        [r["y"].reshape(IMGS_PER_CORE, C, H, W) for r in results], axis=0
    )


def _general_host_path(img, weight, bias):
    """Bit-faithful numpy replica of the reference for arbitrary tables."""
    x = np.transpose(img, (0, 2, 3, 1))
    rgb = (x + np.float32(1.0)) * np.float32(127.5)
    idx = (
        rgb[..., 0] * np.float32(65536.0)
        + rgb[..., 1] * np.float32(256.0)
        + rgb[..., 2]
    ).astype(np.int32)
    y = np.tanh(weight[idx] * x + bias[idx])
    return np.ascontiguousarray(np.transpose(y, (0, 3, 1, 2)).astype(np.float32))


def plan(weight, bias):
    """Pick the device program for these tables.

    Returns (nc, shard_fn, unshard_fn) or None if the host fallback is
    required (LUT rows differ, so the per-pixel gather matters)."""
    weight = np.asarray(weight, dtype=np.float32)
    bias = np.asarray(bias, dtype=np.float32)
    rows_const = (
        (weight.min(axis=0) == weight.max(axis=0)).all()
        and (bias.min(axis=0) == bias.max(axis=0)).all()
    )
    if not rows_const:
        return None
    w0, b0 = weight[0], bias[0]
    if (w0 == w0[0]).all() and (b0 == b0[0]).all():
        return (
            build_nc_flat(w0[0], b0[0]),
            shard_inputs_flat,
            unshard_outputs_flat,
        )
    return build_nc(w0, b0), shard_inputs, unshard_outputs


def kernel(img, weight, bias):
    img = np.ascontiguousarray(np.asarray(img, dtype=np.float32))
    assert img.shape == (B, C, H, W), img.shape

    p = plan(weight, bias)
    if p is None:
        return _general_host_path(img, weight, bias)
    nc, shard_fn, unshard_fn = p

    from concourse.bass_utils import run_bass_kernel_spmd

    res = run_bass_kernel_spmd(nc, shard_fn(img), list(range(N_CORES)))
    return unshard_fn(res.results)
